# revision 8
# baseline (speedup 1.0000x reference)
"""Multi-head attention (B=2, S=2048, D=2048, H=16) on 8 trn2 NeuronCores.

Sharding: tensor-parallel over heads. Core c owns heads [2c, 2c+1]:
  - computes q/k/v projections for its 256 output dims (bf16 matmuls, fp32 PSUM)
  - attention for its 2 heads x 2 batches (transposed-score layout, fused
    softmax denominator, no on-chip transposes)
  - partial output projection  merged_c @ Wo[:, c_slice].T  -> [B, D, S] fp32
Host: sums the 8 partials, transposes, adds bo.

All kernel-side operands are host-pre-transposed so that every matmul operand
already has its contraction dim on SBUF partitions.
"""

import numpy as np

try:
    import concourse.bass as bass  # noqa: F401
except ImportError:  # pragma: no cover - fresh grading dir
    import sys

    sys.path.insert(0, "/opt/trn_rl_repo")

import ml_dtypes

import concourse.bacc as bacc
import concourse.mybir as mybir
import concourse.tile as tile
from concourse.bass_utils import run_bass_kernel_spmd

B, S, D, H = 2, 2048, 2048, 16
HD = D // H  # 128
N_CORES = 8
HPC = H // N_CORES  # heads per core = 2
CD = HPC * HD  # per-core projection dims = 256
TOK = B * S  # 4096

BF16 = mybir.dt.bfloat16
F32 = mybir.dt.float32

TT = 512  # token tile (free dim of most matmuls)
KC = D // 128  # contraction chunks for projections = 16
NB = S // 128  # key blocks per batch = 16
NQ = S // TT  # q tiles per batch = 4
NT = S // TT  # token tiles per batch = 4
SCALE = 1.0 / float(np.sqrt(HD))

Act = mybir.ActivationFunctionType


def build_program():
    nc = bacc.Bacc("TRN2", target_bir_lowering=False, debug=False, num_devices=N_CORES)

    xT = nc.dram_tensor("xT", [D, TOK], BF16, kind="ExternalInput").ap()
    wqT = nc.dram_tensor("wqT", [D, CD], BF16, kind="ExternalInput").ap()
    wkT = nc.dram_tensor("wkT", [D, CD], BF16, kind="ExternalInput").ap()
    wvT = nc.dram_tensor("wvT", [D, CD], BF16, kind="ExternalInput").ap()
    woT = nc.dram_tensor("woT", [CD, D], BF16, kind="ExternalInput").ap()
    bq = nc.dram_tensor("bq", [CD], F32, kind="ExternalInput").ap()
    bk = nc.dram_tensor("bk", [CD], F32, kind="ExternalInput").ap()
    bv = nc.dram_tensor("bv", [CD], F32, kind="ExternalInput").ap()
    out = nc.dram_tensor("out", [B, D, S], F32, kind="ExternalOutput").ap()

    with tile.TileContext(nc) as tc:
        _build_tile(nc, tc, xT, wqT, wkT, wvT, woT, bq, bk, bv, out)

    nc.compile()
    return nc


def _build_tile(nc, tc, xT, wqT, wkT, wvT, woT, bq, bk, bv, out):
    import contextlib

    ctx = contextlib.ExitStack()
    with ctx:
        const = ctx.enter_context(tc.tile_pool(name="const", bufs=1))
        xpool = ctx.enter_context(tc.tile_pool(name="x", bufs=2))
        qkv = ctx.enter_context(tc.tile_pool(name="qkv", bufs=2))
        mt_p = ctx.enter_context(tc.tile_pool(name="mt", bufs=3))
        est_p = ctx.enter_context(tc.tile_pool(name="est", bufs=4))
        small = ctx.enter_context(tc.tile_pool(name="small", bufs=3))
        outp = ctx.enter_context(tc.tile_pool(name="outp", bufs=4))
        # PSUM budget (8 banks): mm 2x2 + acc 2x1 + o 2x1 = 8
        ps_mm = ctx.enter_context(tc.tile_pool(name="ps_mm", bufs=2, space="PSUM"))
        ps_acc = ctx.enter_context(tc.tile_pool(name="ps_acc", bufs=2, space="PSUM"))
        ps_o = ctx.enter_context(tc.tile_pool(name="ps_o", bufs=2, space="PSUM"))

        # ---- resident constants ----
        # staged so the first projection matmuls can start ASAP:
        # wq (by ci-halves), first x tile (by ci-quarters), then the rest.
        xTr = xT.rearrange("(c p) t -> p c t", p=128)
        wq_sb = const.tile([128, KC, CD], BF16, tag="wq")
        wk_sb = const.tile([128, KC, CD], BF16, tag="wk")
        wv_sb = const.tile([128, KC, CD], BF16, tag="wv")
        wo_sb = const.tile([128, HPC, D], BF16, tag="wo")
        wqTr = wqT.rearrange("(c p) m -> p c m", p=128)
        wkTr = wkT.rearrange("(c p) m -> p c m", p=128)
        wvTr = wvT.rearrange("(c p) m -> p c m", p=128)

        xt0 = xpool.tile([128, KC, TT], BF16, tag="xt")
        nc.sync.dma_start(wq_sb[:, 0:4, :], wqTr[:, 0:4, :])
        nc.sync.dma_start(xt0[:, 0:4, :], xTr[:, 0:4, 0:TT])
        nc.sync.dma_start(wq_sb[:, 4:, :], wqTr[:, 4:, :])
        nc.sync.dma_start(xt0[:, 4:, :], xTr[:, 4:, 0:TT])
        nc.sync.dma_start(wk_sb[:], wkTr)
        nc.sync.dma_start(wv_sb[:], wvTr)
        nc.sync.dma_start(wo_sb[:], woT.rearrange("(h p) m -> p h m", p=128))

        bq_sb = const.tile([128, HPC], F32, tag="bq")
        nc.sync.dma_start(bq_sb[:], bq.rearrange("(h p) -> p h", p=128))
        bk_sb = const.tile([128, HPC], F32, tag="bk")
        nc.sync.dma_start(bk_sb[:], bk.rearrange("(h p) -> p h", p=128))
        bv_sb = const.tile([128, HPC], F32, tag="bv")
        nc.sync.dma_start(bv_sb[:], bv.rearrange("(h p) -> p h", p=128))

        ones_sb = const.tile([128, 128], BF16, tag="ones")
        nc.vector.memset(ones_sb[:], 1.0)

        pending_p3 = []  # deferred out-projection emitters (1-qt pipeline)
        states = {}

        def get_state(b):
            if b not in states:
                states[b] = {
                    "QT": qkv.tile([128, HPC, S], BF16, tag="QT", name=f"QT{b}"),
                    "KT": qkv.tile([128, HPC, S], BF16, tag="KT", name=f"KT{b}"),
                    "V": qkv.tile([128, NB, CD], BF16, tag="V", name=f"V{b}"),
                }
            return states[b]

        def p1_tile(b, t):
            st = get_state(b)
            QT, KT, V = st["QT"], st["KT"], st["V"]
            off = b * S + t * TT
            if b == 0 and t == 0:
                xt = xt0
            else:
                xt = xpool.tile([128, KC, TT], BF16, tag="xt")
                nc.sync.dma_start(xt[:], xTr[:, :, off : off + TT])
            for h in range(HPC):
                mo = h * HD
                for w_sb, bias_sb, dst in (
                    (wq_sb, bq_sb, QT),
                    (wk_sb, bk_sb, KT),
                ):
                    p_ps = ps_mm.tile([128, TT], F32, tag="mm")
                    for ci in range(KC):
                        nc.tensor.matmul(
                            p_ps[:],
                            w_sb[:, ci, mo : mo + HD],
                            xt[:, ci, :],
                            start=(ci == 0),
                            stop=(ci == KC - 1),
                        )
                    nc.scalar.activation(
                        dst[:, h, t * TT : (t + 1) * TT],
                        p_ps[:],
                        Act.Identity,
                        bias=bias_sb[:, h : h + 1],
                    )
            for tb in range(TT // 128):
                v_ps = ps_o.tile([128, CD], F32, tag="o")
                for ci in range(KC):
                    nc.tensor.matmul(
                        v_ps[:],
                        xt[:, ci, tb * 128 : (tb + 1) * 128],
                        wv_sb[:, ci, :],
                        start=(ci == 0),
                        stop=(ci == KC - 1),
                    )
                nc.vector.tensor_copy(V[:, t * (TT // 128) + tb, :], v_ps[:])

        # ---- prologue: projections for batch 0 ----
        for t in range(NT):
            p1_tile(0, t)

        for b in range(B):
            QT, KT, V = (get_state(b)[k] for k in ("QT", "KT", "V"))
            # ---- attention per q tile (+interleaved next-batch P1 + P3) ----
            for qt in range(NQ):
                qsl = slice(qt * TT, (qt + 1) * TT)
                MT = mt_p.tile([128, HPC, TT], BF16, tag="MT")  # merged^T slice
                for h in range(HPC):
                    attn_ps = ps_acc.tile([128, TT], F32, tag="acc")
                    dacc = small.tile([128, 2, TT], BF16, tag="dacc")
                    for kp in range(NB // 2):
                        st_ps = ps_mm.tile([128, 2, TT], F32, tag="mm")
                        for j in range(2):
                            kb = 2 * kp + j
                            nc.tensor.matmul(
                                st_ps[:, j, :],
                                KT[:, h, kb * 128 : (kb + 1) * 128],
                                QT[:, h, qsl],
                                start=True,
                                stop=True,
                            )
                        est = est_p.tile([128, 2, TT], BF16, tag="est")
                        nc.scalar.activation(est[:], st_ps[:], Act.Exp, scale=SCALE)
                        if kp == 0:
                            nc.vector.tensor_copy(dacc[:], est[:])
                        else:
                            nc.vector.tensor_add(dacc[:], dacc[:], est[:])
                        for j in range(2):
                            kb = 2 * kp + j
                            nc.tensor.matmul(
                                attn_ps[:],
                                V[:, kb, h * HD : (h + 1) * HD],
                                est[:, j, :],
                                start=(kb == 0),
                                stop=(kb == NB - 1),
                            )
                    dn_ps = ps_o.tile([128, TT], F32, tag="o")
                    nc.tensor.matmul(
                        dn_ps[:], ones_sb[:], dacc[:, 0, :], start=True, stop=False
                    )
                    nc.tensor.matmul(
                        dn_ps[:], ones_sb[:], dacc[:, 1, :], start=False, stop=True
                    )
                    recip = small.tile([128, TT], F32, tag="recip")
                    nc.vector.reciprocal(recip[:], dn_ps[:])
                    t1 = small.tile([128, TT], F32, tag="t1")
                    nc.vector.tensor_mul(t1[:], attn_ps[:], recip[:])
                    nc.scalar.activation(
                        MT[:, h, :], t1[:], Act.Identity, bias=bv_sb[:, h : h + 1]
                    )

                if b + 1 < B:
                    p1_tile(b + 1, qt)
                pending_p3.append(_make_p3(nc, ps_o, outp, wo_sb, MT, out, b, qsl))
                if len(pending_p3) > 1:
                    pending_p3.pop(0)()
        while pending_p3:
            pending_p3.pop(0)()


def _make_p3(nc, ps_o, outp, wo_sb, MT, out, b, qsl):
    def emit():
        for dblk in range(D // 128):
            o_ps = ps_o.tile([128, TT], F32, tag="o")
            for h in range(HPC):
                nc.tensor.matmul(
                    o_ps[:],
                    wo_sb[:, h, dblk * 128 : (dblk + 1) * 128],
                    MT[:, h, :],
                    start=(h == 0),
                    stop=(h == HPC - 1),
                )
            o_sb = outp.tile([128, TT], F32, tag="o")
            if dblk % 3 == 2:
                nc.scalar.copy(o_sb[:], o_ps[:])
            else:
                nc.vector.tensor_copy(o_sb[:], o_ps[:])
            nc.sync.dma_start(out[b, dblk * 128 : (dblk + 1) * 128, qsl], o_sb[:])

    return emit


_program = None


def _get_program():
    global _program
    if _program is None:
        _program = build_program()
    return _program


def kernel(x, Wq, bq, Wk, bk, Wv, bv, Wo, bo):
    x = np.asarray(x, np.float32)
    Wq, Wk, Wv, Wo = (np.asarray(w, np.float32) for w in (Wq, Wk, Wv, Wo))
    bq, bk, bv, bo = (np.asarray(v, np.float32) for v in (bq, bk, bv, bo))

    bf = ml_dtypes.bfloat16
    xT = np.ascontiguousarray(x.reshape(TOK, D).T).astype(bf)

    nc = _get_program()
    in_maps = []
    for c in range(N_CORES):
        sl = slice(c * CD, (c + 1) * CD)
        in_maps.append(
            {
                "xT": xT,
                "wqT": np.ascontiguousarray(Wq[sl, :].T).astype(bf),
                "wkT": np.ascontiguousarray(Wk[sl, :].T).astype(bf),
                "wvT": np.ascontiguousarray(Wv[sl, :].T).astype(bf),
                "woT": np.ascontiguousarray(Wo[:, sl].T).astype(bf),
                "bq": np.ascontiguousarray(bq[sl]),
                "bk": np.ascontiguousarray(bk[sl]),
                "bv": np.ascontiguousarray(bv[sl]),
            }
        )

    res = run_bass_kernel_spmd(nc, in_maps, core_ids=list(range(N_CORES)))
    acc = np.zeros((B, D, S), np.float32)
    for r in res.results:
        acc += r["out"]
    return np.ascontiguousarray(acc.transpose(0, 2, 1)) + bo


# revision 10
# speedup vs baseline: 1.0456x; 1.0456x over previous
"""Multi-head attention (B=2, S=2048, D=2048, H=16) on 8 trn2 NeuronCores.

Sharding: tensor-parallel over heads. Core c owns heads [2c, 2c+1]:
  - computes q/k/v projections for its 256 output dims (bf16 matmuls, fp32 PSUM)
  - attention for its 2 heads x 2 batches (transposed-score layout, fused
    softmax denominator, no on-chip transposes)
  - partial output projection  merged_c @ Wo[:, c_slice].T  -> [B, D, S] fp32
Host: sums the 8 partials, transposes, adds bo.

All kernel-side operands are host-pre-transposed so that every matmul operand
already has its contraction dim on SBUF partitions.
"""

import numpy as np

try:
    import concourse.bass as bass  # noqa: F401
except ImportError:  # pragma: no cover - fresh grading dir
    import sys

    sys.path.insert(0, "/opt/trn_rl_repo")

import ml_dtypes

import concourse.bacc as bacc
import concourse.mybir as mybir
import concourse.tile as tile
from concourse.bass_utils import run_bass_kernel_spmd

B, S, D, H = 2, 2048, 2048, 16
HD = D // H  # 128
N_CORES = 8
HPC = H // N_CORES  # heads per core = 2
CD = HPC * HD  # per-core projection dims = 256
TOK = B * S  # 4096

BF16 = mybir.dt.bfloat16
F32 = mybir.dt.float32

TT = 512  # token tile (free dim of most matmuls)
KC = D // 128  # contraction chunks for projections = 16
NB = S // 128  # key blocks per batch = 16
NQ = S // TT  # q tiles per batch = 4
NT = S // TT  # token tiles per batch = 4
SCALE = 1.0 / float(np.sqrt(HD))

Act = mybir.ActivationFunctionType


def build_program():
    nc = bacc.Bacc("TRN2", target_bir_lowering=False, debug=False, num_devices=N_CORES)

    xT = nc.dram_tensor("xT", [D, TOK], BF16, kind="ExternalInput").ap()
    wqT = nc.dram_tensor("wqT", [D, CD], BF16, kind="ExternalInput").ap()
    wkT = nc.dram_tensor("wkT", [D, CD], BF16, kind="ExternalInput").ap()
    wvT = nc.dram_tensor("wvT", [D, CD], BF16, kind="ExternalInput").ap()
    woT = nc.dram_tensor("woT", [CD, D], BF16, kind="ExternalInput").ap()
    bq = nc.dram_tensor("bq", [CD], F32, kind="ExternalInput").ap()
    bk = nc.dram_tensor("bk", [CD], F32, kind="ExternalInput").ap()
    bv = nc.dram_tensor("bv", [CD], F32, kind="ExternalInput").ap()
    out = nc.dram_tensor("out", [B, D, S], F32, kind="ExternalOutput").ap()

    with tile.TileContext(nc) as tc:
        _build_tile(nc, tc, xT, wqT, wkT, wvT, woT, bq, bk, bv, out)

    nc.compile()
    return nc


def _build_tile(nc, tc, xT, wqT, wkT, wvT, woT, bq, bk, bv, out):
    import contextlib

    ctx = contextlib.ExitStack()
    with ctx:
        const = ctx.enter_context(tc.tile_pool(name="const", bufs=1))
        xpool = ctx.enter_context(tc.tile_pool(name="x", bufs=2))
        qkv = ctx.enter_context(tc.tile_pool(name="qkv", bufs=2))
        mt_p = ctx.enter_context(tc.tile_pool(name="mt", bufs=3))
        est_p = ctx.enter_context(tc.tile_pool(name="est", bufs=4))
        small = ctx.enter_context(tc.tile_pool(name="small", bufs=3))
        outp = ctx.enter_context(tc.tile_pool(name="outp", bufs=4))
        # PSUM budget (8 banks): mm 2x2 + acc 2x1 + o 2x1 = 8
        ps_mm = ctx.enter_context(tc.tile_pool(name="ps_mm", bufs=2, space="PSUM"))
        ps_acc = ctx.enter_context(tc.tile_pool(name="ps_acc", bufs=2, space="PSUM"))
        ps_o = ctx.enter_context(tc.tile_pool(name="ps_o", bufs=2, space="PSUM"))

        # ---- resident constants ----
        # staged so the first projection matmuls can start ASAP:
        # wq (by ci-halves), first x tile (by ci-quarters), then the rest.
        xTr = xT.rearrange("(c p) t -> p c t", p=128)
        wq_sb = const.tile([128, KC, CD], BF16, tag="wq")
        wk_sb = const.tile([128, KC, CD], BF16, tag="wk")
        wv_sb = const.tile([128, KC, CD], BF16, tag="wv")
        wo_sb = const.tile([128, HPC, D], BF16, tag="wo")
        wqTr = wqT.rearrange("(c p) m -> p c m", p=128)
        wkTr = wkT.rearrange("(c p) m -> p c m", p=128)
        wvTr = wvT.rearrange("(c p) m -> p c m", p=128)

        xt0 = xpool.tile([128, KC, TT], BF16, tag="xt")
        nc.sync.dma_start(wq_sb[:, 0:4, :], wqTr[:, 0:4, :])
        nc.sync.dma_start(xt0[:, 0:4, :], xTr[:, 0:4, 0:TT])
        nc.sync.dma_start(wq_sb[:, 4:, :], wqTr[:, 4:, :])
        nc.sync.dma_start(xt0[:, 4:, :], xTr[:, 4:, 0:TT])
        nc.sync.dma_start(wk_sb[:], wkTr)
        nc.sync.dma_start(wv_sb[:], wvTr)
        nc.sync.dma_start(wo_sb[:], woT.rearrange("(h p) m -> p h m", p=128))

        bq_sb = const.tile([128, HPC], F32, tag="bq")
        nc.sync.dma_start(bq_sb[:], bq.rearrange("(h p) -> p h", p=128))
        bk_sb = const.tile([128, HPC], F32, tag="bk")
        nc.sync.dma_start(bk_sb[:], bk.rearrange("(h p) -> p h", p=128))
        bv_sb = const.tile([128, HPC], F32, tag="bv")
        nc.sync.dma_start(bv_sb[:], bv.rearrange("(h p) -> p h", p=128))

        ones_sb = const.tile([128, 128], BF16, tag="ones")
        nc.vector.memset(ones_sb[:], 1.0)

        pending_p3 = []  # deferred out-projection emitters (1-qt pipeline)
        states = {}

        def get_state(b):
            if b not in states:
                states[b] = {
                    "QT": qkv.tile([128, HPC, S], BF16, tag="QT", name=f"QT{b}"),
                    "KT": qkv.tile([128, HPC, S], BF16, tag="KT", name=f"KT{b}"),
                    "V": qkv.tile([128, NB, CD], BF16, tag="V", name=f"V{b}"),
                }
            return states[b]

        def p1_tile(b, t):
            st = get_state(b)
            QT, KT, V = st["QT"], st["KT"], st["V"]
            off = b * S + t * TT
            if b == 0 and t == 0:
                xt = xt0
            else:
                xt = xpool.tile([128, KC, TT], BF16, tag="xt")
                nc.sync.dma_start(xt[:], xTr[:, :, off : off + TT])
            for h in range(HPC):
                mo = h * HD
                for w_sb, bias_sb, dst in (
                    (wq_sb, bq_sb, QT),
                    (wk_sb, bk_sb, KT),
                ):
                    p_ps = ps_mm.tile([128, TT], F32, tag="mm")
                    for ci in range(KC):
                        nc.tensor.matmul(
                            p_ps[:],
                            w_sb[:, ci, mo : mo + HD],
                            xt[:, ci, :],
                            start=(ci == 0),
                            stop=(ci == KC - 1),
                        )
                    nc.scalar.activation(
                        dst[:, h, t * TT : (t + 1) * TT],
                        p_ps[:],
                        Act.Identity,
                        bias=bias_sb[:, h : h + 1],
                    )
            for tb in range(TT // 128):
                v_ps = ps_acc.tile([128, CD], F32, tag="acc")
                for ci in range(KC):
                    nc.tensor.matmul(
                        v_ps[:],
                        xt[:, ci, tb * 128 : (tb + 1) * 128],
                        wv_sb[:, ci, :],
                        start=(ci == 0),
                        stop=(ci == KC - 1),
                    )
                nc.vector.tensor_copy(V[:, t * (TT // 128) + tb, :], v_ps[:])

        for b in range(B):
            for t in range(NT):
                p1_tile(b, t)
            QT, KT, V = (get_state(b)[k] for k in ("QT", "KT", "V"))
            states.pop(b - 1, None)
            # ---- attention per q tile, P3 of previous q tile injected ----
            for qt in range(NQ):
                qsl = slice(qt * TT, (qt + 1) * TT)
                MT = mt_p.tile([128, HPC, TT], BF16, tag="MT")  # merged^T slice
                for h in range(HPC):
                    attn_ps = ps_acc.tile([128, TT], F32, tag="acc")
                    dacc = small.tile([128, 2, TT], BF16, tag="dacc")
                    for kp in range(NB // 2):
                        st_ps = ps_mm.tile([128, 2, TT], F32, tag="mm")
                        for j in range(2):
                            kb = 2 * kp + j
                            nc.tensor.matmul(
                                st_ps[:, j, :],
                                KT[:, h, kb * 128 : (kb + 1) * 128],
                                QT[:, h, qsl],
                                start=True,
                                stop=True,
                            )
                        est = est_p.tile([128, 2, TT], BF16, tag="est")
                        nc.scalar.activation(est[:], st_ps[:], Act.Exp, scale=SCALE)
                        if kp == 0:
                            nc.vector.tensor_copy(dacc[:], est[:])
                        else:
                            nc.vector.tensor_add(dacc[:], dacc[:], est[:])
                        for j in range(2):
                            kb = 2 * kp + j
                            nc.tensor.matmul(
                                attn_ps[:],
                                V[:, kb, h * HD : (h + 1) * HD],
                                est[:, j, :],
                                start=(kb == 0),
                                stop=(kb == NB - 1),
                            )
                        if pending_p3:
                            if next(pending_p3[0], None) is None:
                                pending_p3.pop(0)
                    dn_ps = ps_o.tile([128, TT], F32, tag="o")
                    nc.tensor.matmul(
                        dn_ps[:], ones_sb[:], dacc[:, 0, :], start=True, stop=False
                    )
                    nc.tensor.matmul(
                        dn_ps[:], ones_sb[:], dacc[:, 1, :], start=False, stop=True
                    )
                    recip = small.tile([128, TT], F32, tag="recip")
                    nc.vector.reciprocal(recip[:], dn_ps[:])
                    t1 = small.tile([128, TT], F32, tag="t1")
                    nc.vector.tensor_mul(t1[:], attn_ps[:], recip[:])
                    nc.scalar.activation(
                        MT[:, h, :], t1[:], Act.Identity, bias=bv_sb[:, h : h + 1]
                    )

                pending_p3.append(_p3_steps(nc, ps_o, outp, wo_sb, MT, out, b, qsl))
        while pending_p3:
            for _ in pending_p3.pop(0):
                pass


def _p3_steps(nc, ps_o, outp, wo_sb, MT, out, b, qsl):
    """Generator: one out-projection dblk per next() — injected between
    attention matmul pairs to fill PE gaps."""
    for dblk in range(D // 128):
        o_ps = ps_o.tile([128, TT], F32, tag="o", name=f"o_ps{b}_{dblk}")
        for h in range(HPC):
            nc.tensor.matmul(
                o_ps[:],
                wo_sb[:, h, dblk * 128 : (dblk + 1) * 128],
                MT[:, h, :],
                start=(h == 0),
                stop=(h == HPC - 1),
            )
        o_sb = outp.tile([128, TT], F32, tag="o", name=f"o_sb{b}_{dblk}")
        if dblk % 3 == 2:
            nc.scalar.copy(o_sb[:], o_ps[:])
        else:
            nc.vector.tensor_copy(o_sb[:], o_ps[:])
        nc.sync.dma_start(out[b, dblk * 128 : (dblk + 1) * 128, qsl], o_sb[:])
        yield dblk


_program = None


def _get_program():
    global _program
    if _program is None:
        _program = build_program()
    return _program


def kernel(x, Wq, bq, Wk, bk, Wv, bv, Wo, bo):
    x = np.asarray(x, np.float32)
    Wq, Wk, Wv, Wo = (np.asarray(w, np.float32) for w in (Wq, Wk, Wv, Wo))
    bq, bk, bv, bo = (np.asarray(v, np.float32) for v in (bq, bk, bv, bo))

    bf = ml_dtypes.bfloat16
    xT = np.ascontiguousarray(x.reshape(TOK, D).T).astype(bf)

    nc = _get_program()
    in_maps = []
    for c in range(N_CORES):
        sl = slice(c * CD, (c + 1) * CD)
        in_maps.append(
            {
                "xT": xT,
                "wqT": np.ascontiguousarray(Wq[sl, :].T).astype(bf),
                "wkT": np.ascontiguousarray(Wk[sl, :].T).astype(bf),
                "wvT": np.ascontiguousarray(Wv[sl, :].T).astype(bf),
                "woT": np.ascontiguousarray(Wo[:, sl].T).astype(bf),
                "bq": np.ascontiguousarray(bq[sl]),
                "bk": np.ascontiguousarray(bk[sl]),
                "bv": np.ascontiguousarray(bv[sl]),
            }
        )

    res = run_bass_kernel_spmd(nc, in_maps, core_ids=list(range(N_CORES)))
    acc = np.zeros((B, D, S), np.float32)
    for r in res.results:
        acc += r["out"]
    return np.ascontiguousarray(acc.transpose(0, 2, 1)) + bo
